# revision 30
# baseline (speedup 1.0000x reference)
"""Trainium2 Bass kernel: GSpade node embedding — fp8 DR, dual interleaved scans.

Computation (see reference):
  bidirectional tanh-RNN (512/dir) over 32768 tokens in 2048 sorted ragged
  segments; mean-pool per segment; concat with x @ Wx.T + bx -> [2048, 2048].

Sharding (8 NeuronCores, SPMD): cores 0-3 forward, 4-7 backward.  Segments
sorted by length desc and striped 8 ways; scan core c runs TWO interleaved
sub-scans: alpha = ranks 8i+c, beta = ranks 8i+4+c (256 lanes each).  The two
sub-scans alternate on every engine, so the serial tanh->matmul->tanh latency
of one is hidden under the other's activation work.  Lanes are end-aligned to
shared schedules L_a[i]=len(rank 8i), L_b[i]=len(rank 8i+4); zero-prefixed
lanes keep h==0 until their first token (DR-pair masked bias), so active
lanes are a shrinking prefix -> pure slicing.

Per sub-step (n active lanes), all scan math in fp8e4 DoubleRow (weights
x256, tanh descales):
  - input proj + masked bias: 4 DR matmuls (pair = tokens | mask strip)
  - recurrence: 2 DR passes x 4 out chunks over h~(t-1)
  - tanh: ONE ACT inst [128,4,n] psum->fp8 h~, scale=1/256
  - h~ ring of 4 slots so pooled-accumulate reads never gate the next tanh
Pooling: DVE (chunks 0-2) and GpSimd (chunk 3) accumulate the fp8 h~ ring
directly into an SBUF fp32 accumulator each sub-step (reads cover exactly the
freshly written width); retired lanes are finalized (x 1/len) and DMA'd out
in stages.
PSUM: one pool of [128,2048] tiles (4 banks) x bufs=2 — the alpha/beta
alternation itself provides double buffering.
x-projection (bf16, transposed, bias via DVE tensor_scalar, bf16 output)
runs at the tail through the same psum pool.
"""

import ml_dtypes
import numpy as np

import concourse.bacc as bacc
import concourse.mybir as mybir
from concourse.tile import TileContext
from concourse.bass_utils import run_bass_kernel_spmd

FP32 = mybir.dt.float32
BF16 = mybir.dt.bfloat16
FP8 = mybir.dt.float8e4
DR = mybir.MatmulPerfMode.DoubleRow
Tanh = mybir.ActivationFunctionType.Tanh
E4 = ml_dtypes.float8_e4m3

N_GROUPS = 2048
D_SEQ = 128
H = 512
HC = 4
D_PROJ = 1024
N_CORES = 8
LANES = 256       # per sub-scan
XROWS = N_GROUPS // N_CORES
SCALE = 256.0

_program_cache: dict = {}


def _dma_chunks(nt, target=1536):
    # fine-grained early boundaries so the first steps are never starved,
    # coarse afterwards to limit descriptor count
    bounds = {1, 2, 4}
    chunks = []
    t0 = 0
    cols = 0
    for t, n in enumerate(nt):
        if t in bounds or (cols > 0 and cols + n > target):
            chunks.append((t0, t))
            t0, cols = t, 0
        cols += n
    chunks.append((t0, len(nt)))
    return chunks


def _build_program(nta, ntb):
    nta, ntb = list(nta), list(ntb)
    sa, sb_ = len(nta), len(ntb)
    offa = np.concatenate([[0], np.cumsum(nta)]).astype(int)
    offb = np.concatenate([[0], np.cumsum(ntb)]).astype(int)
    Sa, Sb = int(offa[-1]), int(offb[-1])
    Spa = (Sa + 15) // 16 * 16
    Spb = (Sb + 15) // 16 * 16

    nc = bacc.Bacc("TRN2", target_bir_lowering=False, debug=False,
                   num_devices=N_CORES)

    xta_d = nc.dram_tensor("xta", [128, 2 * Spa], FP8, kind="ExternalInput")
    xtb_d = nc.dram_tensor("xtb", [128, 2 * Spb], FP8, kind="ExternalInput")
    wih_d = nc.dram_tensor("wih", [128, 2 * H + 256], FP8, kind="ExternalInput")
    whp_d = nc.dram_tensor("whp", [128, 4 * H], FP8, kind="ExternalInput")
    invbx_d = nc.dram_tensor("invbx", [128, 2 * LANES + 8], FP32, kind="ExternalInput")
    xT_d = nc.dram_tensor("xT", [128, 4 * XROWS], BF16, kind="ExternalInput")
    wxT_d = nc.dram_tensor("wxT", [128, 4 * D_PROJ], BF16, kind="ExternalInput")

    xpT_d = nc.dram_tensor("xpT", [D_PROJ, XROWS], BF16, kind="ExternalOutput")
    # pooled: alpha lanes in cols [0,256), beta in [256,512)
    pooledT_d = nc.dram_tensor("pooledT", [H, 2 * LANES], FP32, kind="ExternalOutput")

    with TileContext(nc) as tc:
        with (
            tc.tile_pool(name="sb", bufs=1) as sb,
            tc.tile_pool(name="ps", bufs=2, space="PSUM") as psp,
        ):
            # ---- SBUF tiles + merged, head-ordered DMA ----
            xta_sb = sb.tile([128, 2 * Spa], FP8, tag="xta", name="xta")
            xtb_sb = sb.tile([128, 2 * Spb], FP8, tag="xtb", name="xtb")
            x3a = xta_sb.rearrange("p (i s) -> p i s", i=2)
            x3b = xtb_sb.rearrange("p (i s) -> p i s", i=2)
            d3a = xta_d.rearrange("p (i s) -> p i s", i=2)
            d3b = xtb_d.rearrange("p (i s) -> p i s", i=2)
            wih_sb = sb.tile([128, 2 * H + 256], FP8, tag="wih", name="wih")
            whp_sb = sb.tile([128, 4 * H], FP8, tag="whp", name="whp")
            invbx_sb = sb.tile([128, 2 * LANES + 8], FP32, tag="invbx", name="invbx")
            wx_sb = sb.tile([128, 4 * D_PROJ], BF16, tag="wx", name="wx")
            xT_sb = sb.tile([128, 4 * XROWS], BF16, tag="xT", name="xT")

            cha = _dma_chunks(nta)
            chb = _dma_chunks(ntb)

            def dma_tok(which, idx):
                if which == 0 and idx < len(cha):
                    t0, t1 = cha[idx]
                    a, b = int(offa[t0]), int(offa[t1])
                    nc.sync.dma_start(out=x3a[:, :, a:b], in_=d3a[:, :, a:b])
                if which == 1 and idx < len(chb):
                    t0, t1 = chb[idx]
                    a, b = int(offb[t0]), int(offb[t1])
                    nc.sync.dma_start(out=x3b[:, :, a:b], in_=d3b[:, :, a:b])

            nc.sync.dma_start(out=wih_sb[:, :], in_=wih_d[:, :])
            dma_tok(0, 0)
            dma_tok(1, 0)
            nc.sync.dma_start(out=whp_sb[:, :], in_=whp_d[:, :])
            dma_tok(0, 1)
            dma_tok(1, 1)
            dma_tok(0, 2)
            dma_tok(1, 2)
            nc.sync.dma_start(out=invbx_sb[:, :], in_=invbx_d[:, :])
            for i in range(3, max(len(cha), len(chb))):
                dma_tok(0, i)
                dma_tok(1, i)
            nc.sync.dma_start(out=wx_sb[:, :], in_=wxT_d[:, :])
            nc.sync.dma_start(out=xT_sb[:, :], in_=xT_d[:, :])

            # h~ rings: [p, slot(4), chunk(4), lane(256)] per sub-scan
            h2a = sb.tile([128, 4 * HC * LANES], FP8, tag="h2a", name="h2a")
            h2b = sb.tile([128, 4 * HC * LANES], FP8, tag="h2b", name="h2b")
            h4a = h2a.rearrange("p (s c j) -> p s c j", s=4, c=HC)
            h4b = h2b.rearrange("p (s c j) -> p s c j", s=4, c=HC)
            ip3 = wih_sb[:, 2 * H:2 * H + 256].rearrange("p (i o) -> p i o", i=2)

            # SBUF pooled accumulators (fp32): [p, chunk(4), lane(256)] x2
            acc_sb = sb.tile([128, 2 * HC * LANES], FP32, tag="acc", name="acc")
            ac4 = acc_sb.rearrange("p (x c j) -> p x c j", x=2, c=HC)
            nc.vector.memset(ac4[:, 0], 0.0)
            nc.gpsimd.memset(ac4[:, 1], 0.0)

            po_sb = sb.tile([128, HC * 2 * LANES], FP32, tag="po", name="po")
            po4 = po_sb.rearrange("p (c j) -> p c j", c=HC)    # j in [0,512)
            pd3 = pooledT_d.rearrange("(c p) j -> p c j", c=HC)

            xpo_sb = sb.tile([128, 8 * XROWS], BF16, tag="xpo", name="xpo")
            xpd3 = xpT_d.rearrange("(g p) j -> p g j", g=8)
            xpo3 = xpo_sb.rearrange("p (g j) -> p g j", g=8)

            # ACT tanh table pre-warm
            warm_sb = sb.tile([128, 2], FP32, tag="warm", name="warm")
            nc.vector.memset(warm_sb[:, :], 0.0)
            nc.scalar.activation(warm_sb[:, :], warm_sb[:, :], Tanh)

            wih3 = wih_sb[:, 0:2 * H].rearrange("p (i o) -> p i o", i=2)
            wh4 = whp_sb.rearrange("p (k i o) -> p k i o", k=2, i=2)

            def substep(t, nt, off, x3, h4):
                n = nt[t]
                a = int(off[t])
                s = t % 4
                r = (t - 1) % 4
                ps = psp.tile([128, HC * 512], FP32, tag="ps", name="ps")
                ps3 = ps.rearrange("p (c j) -> p c j", c=HC)
                for jc in range(HC):
                    nc.tensor.matmul(
                        ps[:, jc * 512:jc * 512 + n],
                        wih3[:, :, jc * 128:(jc + 1) * 128],
                        x3[:, :, a:a + n],
                        start=True, stop=(t == 0), perf_mode=DR)
                if t > 0:
                    for kc2 in range(2):
                        for jc in range(HC):
                            nc.tensor.matmul(
                                ps[:, jc * 512:jc * 512 + n],
                                wh4[:, kc2, :, jc * 128:(jc + 1) * 128],
                                h4[:, r, 2 * kc2:2 * kc2 + 2, 0:n],
                                start=False, stop=(kc2 == 1), perf_mode=DR)
                nc.scalar.activation(h4[:, s, :, 0:n], ps3[:, :, 0:n],
                                     Tanh, scale=1.0 / SCALE)

            def accum(t, nt, h4, x):
                """pooled accumulate: acc += h~(t), chunks 0-2 on DVE,
                chunk 3 on Pool; reads exactly the freshly written width."""
                n = nt[t]
                s = t % 4
                nc.vector.tensor_add(ac4[:, x, 0:3, 0:n],
                                     ac4[:, x, 0:3, 0:n],
                                     h4[:, s, 0:3, 0:n])
                nc.gpsimd.tensor_add(ac4[:, x, 3, 0:n],
                                     ac4[:, x, 3, 0:n],
                                     h4[:, s, 3, 0:n])

            def fin_cols(x, lo, hi):
                """finalize pooled cols [lo,hi) of sub-scan x (acc * 1/len)."""
                g = x * LANES
                for c in range(HC):
                    eng = nc.vector if c < 2 else nc.gpsimd
                    eng.tensor_mul(po4[:, c, g + lo:g + hi],
                                   ac4[:, x, c, lo:hi],
                                   invbx_sb[:, g + lo:g + hi])
                nc.sync.dma_start(out=pd3[:, :, g + lo:g + hi],
                                  in_=po4[:, :, g + lo:g + hi])

            def xproj_group(pc):
                xps = psp.tile([128, HC * 512], FP32, tag="ps", name="xps")
                o = xps[:, 0:XROWS]
                for kc in range(4):
                    nc.tensor.matmul(
                        o,
                        wx_sb[:, kc * D_PROJ + pc * 128:kc * D_PROJ + (pc + 1) * 128],
                        xT_sb[:, kc * XROWS:(kc + 1) * XROWS],
                        start=(kc == 0), stop=(kc == 3))
                nc.vector.tensor_scalar_add(
                    xpo3[:, pc, :], o,
                    invbx_sb[:, 2 * LANES + pc:2 * LANES + pc + 1])
                if pc % 2 == 1:
                    nc.sync.dma_start(out=xpd3[:, pc - 1:pc + 1, :],
                                      in_=xpo3[:, pc - 1:pc + 1, :])

            # staged finalize thresholds (per sub-scan)
            fina = {}
            finb = {}
            for fin, nt, hi0 in ((fina, nta, LANES), (finb, ntb, LANES)):
                done = hi0
                for u in range(1, len(nt), 2):
                    nxt = nt[u + 1] if u + 1 < len(nt) else 0
                    if done - nxt >= 40 and done > 40:
                        fin[u] = (nxt, done)
                        done = nxt
                fin["end"] = (0, done)

            # ---- interleaved dual scan ----
            for t in range(sa):
                substep(t, nta, offa, x3a, h4a)
                if t < sb_:
                    substep(t, ntb, offb, x3b, h4b)
                accum(t, nta, h4a, 0)
                if t < sb_:
                    accum(t, ntb, h4b, 1)
                if t in fina:
                    fin_cols(0, *fina[t])
                if t < sb_ and t in finb:
                    fin_cols(1, *finb[t])
            fin_cols(0, *fina["end"])
            fin_cols(1, *finb["end"])
            for g in range(8):
                xproj_group(g)

    nc.compile()
    return nc


def _get_program(nta, ntb=None):
    if ntb is None:
        nta, ntb = nta
    key = (tuple(nta), tuple(ntb))
    if key not in _program_cache:
        _program_cache[key] = _build_program(nta, ntb)
    return _program_cache[key]


def _prepare(x, seqs, masks, W_ih_f, W_hh_f, b_f, W_ih_b, W_hh_b, b_b, Wx, bx):
    x = np.asarray(x, np.float32)
    seqs = np.asarray(seqs, np.float32)
    masks = np.asarray(masks).astype(np.int64)

    lens = np.bincount(masks, minlength=N_GROUPS).astype(np.int64)
    starts_all = np.concatenate([[0], np.cumsum(lens)[:-1]])
    order = np.argsort(-lens, kind="stable")
    sl = lens[order]

    seqs_pad = np.vstack([np.zeros((1, D_SEQ), np.float32), seqs])

    def schedule(base):
        L = sl[base::8].astype(np.int64)              # 256 lanes
        steps = int(L[0])
        nt = [int((L > t).sum()) for t in range(steps)]
        off = np.concatenate([[0], np.cumsum(nt)]).astype(int)
        return L, steps, nt, off

    La, sa, nta, offa = schedule(0)
    Lb, sb_, ntb, offb = schedule(4)
    Spa = (int(offa[-1]) + 15) // 16 * 16
    Spb = (int(offb[-1]) + 15) // 16 * 16

    def streams(base, L, steps, nt, off, Sp, cidx):
        """token+strip stream for stripe (base+cidx) with schedule L."""
        S = int(off[-1])
        t_grid = np.arange(steps)[:, None]
        active = t_grid < L[None, :]
        g = order[base + cidx::8]
        lens_c = lens[g]
        starts_c = starts_all[g]
        pre = (L - lens_c)[None, :]
        real = active & (t_grid >= pre)
        pos = t_grid - pre
        idx_f = np.where(real, starts_c[None, :] + pos, -1)
        idx_b = np.where(real, starts_c[None, :] + lens_c[None, :] - 1 - pos, -1)
        real_flat = real[active].astype(np.float32)
        xf = np.zeros((128, Sp), E4)
        xb = np.zeros((128, Sp), E4)
        xf[:, :S] = seqs_pad[idx_f[active] + 1].T.astype(E4)
        xb[:, :S] = seqs_pad[idx_b[active] + 1].T.astype(E4)
        strip = np.zeros((128, Sp), E4)
        strip[0, :S] = real_flat.astype(E4)
        return (np.hstack([xf, strip]), np.hstack([xb, strip]), g, lens_c)

    ip = np.zeros((128, 2, 128), E4)
    for k in range(128):
        ip[k, 0, k] = 1.0
        ip[k, 1, k] = 1.0
    ip = ip.reshape(128, 256)

    def pack_wih(W_ih, b):
        Wq = (np.asarray(W_ih, np.float32) * SCALE).astype(E4)
        bq = (np.asarray(b, np.float32) * SCALE).astype(E4)
        out = np.zeros((128, 2 * H + 256), E4)
        out[:, 0:H] = Wq.T
        out[0, H:2 * H] = bq
        out[:, 2 * H:] = ip
        return out

    def pack_whh(W_hh):
        Wq = (np.asarray(W_hh, np.float32) * SCALE).astype(E4)
        WqT = Wq.T
        out = np.zeros((128, 4 * H), E4)
        for kc2 in range(2):
            for i in range(2):
                out[:, (kc2 * 2 + i) * H:(kc2 * 2 + i + 1) * H] = \
                    WqT[kc2 * 256 + i * 128: kc2 * 256 + (i + 1) * 128, :]
        return out

    wxT = np.asarray(Wx, np.float32).T.astype(ml_dtypes.bfloat16)
    wx_m = np.zeros((128, 4 * D_PROJ), ml_dtypes.bfloat16)
    for kc in range(4):
        wx_m[:, kc * D_PROJ:(kc + 1) * D_PROJ] = wxT[kc * 128:(kc + 1) * 128, :]
    bxa = np.asarray(bx, np.float32)

    wihp_f = pack_wih(W_ih_f, b_f)
    wihp_b = pack_wih(W_ih_b, b_b)
    whp_f = pack_whh(W_hh_f)
    whp_b = pack_whh(W_hh_b)

    in_maps = []
    gids = []
    for c4 in range(4):
        xa_f, xa_b, ga, lens_a = streams(0, La, sa, nta, offa, Spa, c4)
        xb_f, xb_b, gb, lens_b = streams(4, Lb, sb_, ntb, offb, Spb, c4)
        gids.append((ga, gb))
        invbx = np.zeros((128, 2 * LANES + 8), np.float32)
        invbx[:, :LANES] = (1.0 / lens_a).astype(np.float32)[None, :]
        invbx[:, LANES:2 * LANES] = (1.0 / lens_b).astype(np.float32)[None, :]
        for pc in range(8):
            invbx[:, 2 * LANES + pc] = bxa[pc * 128:(pc + 1) * 128]
        for fwd in (True, False):
            core = c4 if fwd else c4 + 4
            xTc = x[core * XROWS:(core + 1) * XROWS, :].T.astype(ml_dtypes.bfloat16)
            xT_m = np.zeros((128, 4 * XROWS), ml_dtypes.bfloat16)
            for kc in range(4):
                xT_m[:, kc * XROWS:(kc + 1) * XROWS] = xTc[kc * 128:(kc + 1) * 128, :]
            in_maps.append((core, {
                "xta": xa_f if fwd else xa_b,
                "xtb": xb_f if fwd else xb_b,
                "wih": wihp_f if fwd else wihp_b,
                "whp": whp_f if fwd else whp_b,
                "invbx": invbx,
                "xT": xT_m,
                "wxT": wx_m,
            }))
    in_maps.sort(key=lambda kv: kv[0])
    in_maps = [m for _, m in in_maps]

    return ((nta, ntb), (nta, ntb)), in_maps, gids


def _assemble(res, gids):
    out = np.empty((N_GROUPS, 2 * D_PROJ), np.float32)
    for core in range(N_CORES):
        out[core * XROWS:(core + 1) * XROWS, :D_PROJ] = res[core]["xpT"].T.astype(np.float32)
    for c4 in range(4):
        ga, gb = gids[c4]
        pf = res[c4]["pooledT"]
        pb = res[c4 + 4]["pooledT"]
        out[ga, D_PROJ:D_PROJ + H] = pf[:, :LANES].T
        out[gb, D_PROJ:D_PROJ + H] = pf[:, LANES:].T
        out[ga, D_PROJ + H:] = pb[:, :LANES].T
        out[gb, D_PROJ + H:] = pb[:, LANES:].T
    return out


def kernel(**inputs):
    (ntab, _), in_maps, gids = _prepare(**inputs)
    nc = _get_program(ntab)
    res = run_bass_kernel_spmd(nc, in_maps, list(range(N_CORES))).results
    return _assemble(res, gids)
